# revision 7
# baseline (speedup 1.0000x reference)
"""Trainium2 Bass kernel for nn_Decoder (2-layer masked-skip LSTM decoder).

Strategy: 8-way tensor parallelism over the 4H gate dimension of both LSTM
layers.  Each core owns 128 hidden units per layer (512 gate rows in [i|f|o|g]
order).  The sequential recurrence runs as T+1 "supersteps": superstep s
computes layer-0 step s and layer-1 step s-1, then one fused SBUF->SBUF
remote-DMA broadcast distributes [h0T(s) | h1T(s-1)] slices to all 8 cores
(collective AllGather fallback available).  The input projection
Xp = x @ W_ih0.T (phase A) is precomputed/interleaved as PE filler, and the
output projection y = h1 @ W_out.T (phase C) runs in an epilogue, time-sharded
across cores.  Gate matmuls stream weights as the fp32r moving operand
(N=512 -> 1 row/cycle); skip matmuls use bf16.
"""
import os
import time
from contextlib import ExitStack

import numpy as np
import ml_dtypes

import concourse.bass as bass
import concourse.bacc as bacc
import concourse.mybir as mybir
import concourse.tile as tile
from concourse.tile_rust import add_dep_helper

F32 = mybir.dt.float32
F32R = mybir.dt.float32r
BF16 = mybir.dt.bfloat16
AF = mybir.ActivationFunctionType
ALU = mybir.AluOpType

NCORES = 8
B = 64          # batch
H = 1024        # hidden
IN0 = 1536      # layer-0 input size
OUT = 512       # output size
HS = 128        # hidden slice per core
GS = 512        # gate slice per core (4*HS)
# reference gate order is [i, f, g, o]; we use [i, f, o, g]
GATE_PERM = [0, 1, 3, 2]

_BUILD_CACHE = {}


def build(T, mode="remote"):
    """Build the SPMD program for sequence length T."""
    nc = bacc.Bacc("TRN2", target_bir_lowering=False, debug=False,
                   num_devices=NCORES)
    NCH = 8  # hidden chunks (K chunks of 128)

    # ---------------- I/O ----------------
    tgt = nc.dram_tensor("tgt", [T, B, IN0], F32, kind="ExternalInput")
    Whh0T = nc.dram_tensor("Whh0T", [128, NCH * GS], F32R, kind="ExternalInput")
    Wih1T = nc.dram_tensor("Wih1T", [128, NCH * GS], F32R, kind="ExternalInput")
    Whh1T = nc.dram_tensor("Whh1T", [128, NCH * GS], F32R, kind="ExternalInput")
    Wih0T = nc.dram_tensor("Wih0T", [128, 12 * GS], F32R, kind="ExternalInput")
    WoutT = nc.dram_tensor("WoutT", [128, NCH * OUT], F32R, kind="ExternalInput")
    Wh20b = nc.dram_tensor("Wh20b", [128, NCH * HS], BF16, kind="ExternalInput")
    Wh21b = nc.dram_tensor("Wh21b", [128, NCH * HS], BF16, kind="ExternalInput")
    b0bc = nc.dram_tensor("b0bc", [128, GS], F32, kind="ExternalInput")
    b1bc = nc.dram_tensor("b1bc", [B, GS], F32R, kind="ExternalInput")
    bh20 = nc.dram_tensor("bh20", [B, HS], F32, kind="ExternalInput")
    bh21 = nc.dram_tensor("bh21", [B, HS], F32, kind="ExternalInput")
    bobc = nc.dram_tensor("bobc", [128, OUT], F32R, kind="ExternalInput")
    I64d = nc.dram_tensor("I64d", [64, 64], F32, kind="ExternalInput")
    I64r_d = nc.dram_tensor("I64r_d", [64, 64], F32R, kind="ExternalInput")
    I128d = nc.dram_tensor("I128d", [128, 128], F32, kind="ExternalInput")
    I128r_d = nc.dram_tensor("I128r_d", [128, 128], F32R, kind="ExternalInput")
    onesd = nc.dram_tensor("onesd", [1, 64], BF16, kind="ExternalInput")
    h0Ti = nc.dram_tensor("h0Ti", [128, NCH * 64], F32R, kind="ExternalInput")
    h1Ti = nc.dram_tensor("h1Ti", [128, NCH * 64], F32R, kind="ExternalInput")
    h0Tib = nc.dram_tensor("h0Tib", [128, NCH * 64], BF16, kind="ExternalInput")
    h1Tib = nc.dram_tensor("h1Tib", [128, NCH * 64], BF16, kind="ExternalInput")
    c00 = nc.dram_tensor("c00", [B, HS], F32, kind="ExternalInput")
    c01 = nc.dram_tensor("c01", [B, HS], F32, kind="ExternalInput")
    maskd = nc.dram_tensor("maskd", [1, T * 512], BF16, kind="ExternalInput")

    NT = T // NCORES  # own time-slice length
    y_out = nc.dram_tensor("y_out", [NT, B, OUT], F32, kind="ExternalOutput")

    # scratch DRAM
    Xp_dram = nc.dram_tensor("Xp_dram", [T, B, GS], F32R)
    H1T_dram = nc.dram_tensor("H1T_dram", [T, 128, GS], F32R)

    S = T + 1  # supersteps 0..T

    # ------------- raw prologue -------------
    remote = (mode == "remote")
    if remote:
        nc.gpsimd.bir_kernel_barrier_wait([list(range(NCORES))])
    rank = nc.gpsimd.partition_id()

    patches = []   # (wait_inst, value)
    sem_ctx = ExitStack()
    if remote:
        rem_sem = sem_ctx.enter_context(nc.semaphore("rem_sem"))
        loc_sem = sem_ctx.enter_context(nc.semaphore("loc_sem"))

    MB = 32  # mask staging block (supersteps per block)

    with tile.TileContext(nc) as tc, ExitStack() as ctx:
        sb = ctx.enter_context(tc.tile_pool(name="sb", bufs=1))
        ps = ctx.enter_context(tc.tile_pool(name="ps", bufs=1, space="PSUM"))
        dr = ctx.enter_context(tc.tile_pool(name="dr", bufs=1, space="DRAM"))

        # ---------- load weights (host supplies fp32r directly) ----------
        def load_direct(name, src, cols, dt=F32R):
            res = sb.tile([128, cols], dt, tag=f"{name}_r", name=f"{name}_r")
            nc.sync.dma_start(res[:], src[:, :])
            return res

        whh0 = load_direct("whh0", Whh0T, NCH * GS)
        wih1 = load_direct("wih1", Wih1T, NCH * GS)
        whh1 = load_direct("whh1", Whh1T, NCH * GS)
        wih0 = load_direct("wih0", Wih0T, 12 * GS)
        wout = load_direct("wout", WoutT, NCH * OUT)

        wh20 = sb.tile([128, NCH * HS], BF16, tag="wh20")
        wh21 = sb.tile([128, NCH * HS], BF16, tag="wh21")
        nc.sync.dma_start(wh20[:], Wh20b[:, :])
        nc.sync.dma_start(wh21[:], Wh21b[:, :])

        b0_sb = sb.tile([128, GS], F32, tag="b0_sb")
        nc.sync.dma_start(b0_sb[:], b0bc[:, :])
        b1_r = sb.tile([B, GS], F32R, tag="b1_r")
        nc.sync.dma_start(b1_r[:], b1bc[:, :])
        bo_r = sb.tile([128, OUT], F32R, tag="bo_r")
        nc.sync.dma_start(bo_r[:], bobc[:, :])
        bh20_sb = sb.tile([B, HS], F32, tag="bh20_sb")
        bh21_sb = sb.tile([B, HS], F32, tag="bh21_sb")
        nc.sync.dma_start(bh20_sb[:], bh20[:, :])
        nc.sync.dma_start(bh21_sb[:], bh21[:, :])

        i64f = sb.tile([64, 64], F32, tag="i64f")
        nc.sync.dma_start(i64f[:], I64d[:, :])
        i64r = sb.tile([64, 64], F32R, tag="i64r")
        nc.sync.dma_start(i64r[:], I64r_d[:, :])
        i128f = sb.tile([128, 128], F32, tag="i128f")
        nc.sync.dma_start(i128f[:], I128d[:, :])
        i128r = sb.tile([128, 128], F32R, tag="i128r")
        nc.sync.dma_start(i128r[:], I128r_d[:, :])
        ones_sb = sb.tile([1, 64], BF16, tag="ones_sb")
        nc.sync.dma_start(ones_sb[:], onesd[:, :])

        h0i_r = sb.tile([128, NCH * 64], F32R, tag="h0i_r")
        h1i_r = sb.tile([128, NCH * 64], F32R, tag="h1i_r")
        nc.sync.dma_start(h0i_r[:], h0Ti[:, :])
        nc.sync.dma_start(h1i_r[:], h1Ti[:, :])
        h0i_b = sb.tile([128, NCH * 64], BF16, tag="h0i_b")
        h1i_b = sb.tile([128, NCH * 64], BF16, tag="h1i_b")
        nc.sync.dma_start(h0i_b[:], h0Tib[:, :])
        nc.sync.dma_start(h1i_b[:], h1Tib[:, :])

        zb = sb.tile([128, NCH * 64], BF16, tag="zb")
        nc.vector.memset(zb[:], 0.0)

        c0_st = sb.tile([B, HS], F32, tag="c0_st")
        c1_st = sb.tile([B, HS], F32, tag="c1_st")
        nc.sync.dma_start(c0_st[:], c00[:, :])
        nc.sync.dma_start(c1_st[:], c01[:, :])

        # ---------- state tiles ----------
        hb = [sb.tile([128, 1024], F32R, tag=f"hb{i}", name=f"hb{i}")
              for i in range(3)]
        hbb = [sb.tile([128, 1024], BF16, tag=f"hbb{i}", name=f"hbb{i}")
               for i in range(3)]
        pr = [sb.tile([128, 128], F32R, tag=f"pr{i}", name=f"pr{i}")
              for i in range(3)]
        xp_sb = [sb.tile([B, GS], F32R, tag=f"xp{i}", name=f"xp{i}")
                 for i in range(3)]
        mstage = [sb.tile([1, 512], BF16, tag=f"mst{i}", name=f"mst{i}")
                  for i in range(3)]
        g0_sb = sb.tile([B, GS], F32, tag="g0_sb")
        g1_sb = sb.tile([B, GS], F32, tag="g1_sb")
        cell_t = {n: sb.tile([B, HS], F32, tag=n, name=n)
                  for n in ["t1a", "t2a", "tha", "hca", "u1a", "u2a", "hn0",
                            "t1b", "t2b", "thb", "hcb", "u1b", "u2b", "hn1",
                            "sk0t", "sk1t", "sk0s", "sk1s"]}
        xrow = [sb.tile([128, IN0], F32, tag=f"xrow{i}", name=f"xrow{i}")
                for i in range(2)]
        xT_sb = [sb.tile([128, 12 * 128], F32R, tag=f"xT{i}", name=f"xT{i}")
                 for i in range(2)]
        xp_ev = [sb.tile([128, GS], F32R, tag=f"xpe{i}", name=f"xpe{i}")
                 for i in range(2)]

        # ---------- PSUM ----------
        psG0 = ps.tile([B, GS], F32, tag="psG0")
        psG1 = ps.tile([B, GS], F32, tag="psG1")
        psSK = ps.tile([B, 256], F32, tag="psSK")
        psM = ps.tile([B, GS], F32, tag="psM")
        psT = ps.tile([128, 128], F32, tag="psT")
        psA = [ps.tile([128, GS], F32, tag=f"psA{i}", name=f"psA{i}")
               for i in range(2)]
        psX = ps.tile([128, 128], F32, tag="psX")

        nc.vector.memset(psT[:], 0.0)

        if not remote:
            cc_in = [dr.tile([128, 128], F32R, tag=f"cci{i}", name=f"cci{i}")
                     for i in range(2)]
            cc_out = [dr.tile([128 * NCORES, 128], F32R, tag=f"cco{i}",
                              name=f"cco{i}") for i in range(2)]

        # ---------- phase A chunk emitter ----------
        NCHUNK = T // 2  # 2 timesteps per 128-row chunk

        def phase_a_chunk(c):
            par = c % 2
            nc.sync.dma_start(
                xrow[par][:], tgt[bass.ts(c, 2)].rearrange("t b d -> (t b) d"))
            for k in range(12):
                nc.tensor.transpose(psX[:], xrow[par][:, bass.ts(k, 128)],
                                    i128f[:])
                nc.scalar.copy(xT_sb[par][:, bass.ts(k, 128)], psX[:])
            for k in range(12):
                nc.tensor.matmul(psA[par][:],
                                 xT_sb[par][:, bass.ts(k, 128)],
                                 wih0[:, bass.ts(k, GS)],
                                 start=(k == 0), stop=(k == 11))
            # evac with bias0 fold; fp32r rounding happens here
            nc.vector.scalar_tensor_tensor(
                xp_ev[par][:], psA[par][:], 1.0, b0_sb[:],
                ALU.mult, ALU.add)
            nc.sync.dma_start(
                Xp_dram[bass.ts(c, 2)].rearrange("t b d -> (t b) d"),
                xp_ev[par][:])

        LOOKAHEAD = 8
        for c in range(min(LOOKAHEAD, NCHUNK)):
            phase_a_chunk(c)

        # xp prefetch prologue (rows 0, 1)
        def xp_load(s):
            return nc.sync.dma_start(xp_sb[s % 3][:], Xp_dram[s])

        # (prologue loads run before any SP waits; order chain starts here)

        # mask staging
        def mask_load(t):
            if t >= T:
                return None
            return nc.sync.dma_start(
                mstage[t % 3][:], maskd[:, t * 512: (t + 1) * 512])



        # ---------- superstep loop ----------
        sp_chain = [None]   # total order over SP (DMA issue) stream
        pe_anchor = [None]  # last transpose of previous superstep
        dve_anchor = [None]  # pr copy of previous superstep

        def sp_emit(bi):
            if sp_chain[0] is not None:
                add_dep_helper(bi.ins, sp_chain[0].ins, False, "sp order")
            sp_chain[0] = bi
            return bi

        def pwait(eng, sem, val, anchor):
            wi = eng.wait_ge(sem, 0)
            if anchor is not None:
                add_dep_helper(wi.ins, anchor.ins, False, "chain")
            patches.append((wi, val))
            return wi

        sp_emit(xp_load(0))
        if T > 1:
            sp_emit(xp_load(1))
        sp_emit(mask_load(0))
        mi = mask_load(1)
        if mi is not None:
            sp_emit(mi)

        def h0stat(s, c):
            if s == 0:
                return h0i_r[:, bass.ts(c, 64)]
            return hb[(s - 1) % 3][:, c * 128: c * 128 + 64]

        def h1stat(s, c):  # h1(s-2) for L1 h-part at superstep s
            if s == 1:
                return h1i_r[:, bass.ts(c, 64)]
            return hb[(s - 1) % 3][:, c * 128 + 64: c * 128 + 128]

        def sk0stat(s, c):  # h0(s-2) bf16
            if s == 0:
                return zb[:, bass.ts(c, 64)]
            if s == 1:
                return h0i_b[:, bass.ts(c, 64)]
            return hbb[(s - 2) % 3][:, c * 128: c * 128 + 64]

        def sk1stat(s, c):  # h1(s-3) bf16 (L1 skip at step s-1)
            if s == 1:
                return zb[:, bass.ts(c, 64)]
            if s == 2:
                return h1i_b[:, bass.ts(c, 64)]
            return hbb[(s - 2) % 3][:, c * 128 + 64: c * 128 + 128]

        def mrhs(t, layer):
            return mstage[t % 3][:, layer * 256: layer * 256 + 256]

        ct = cell_t

        for s in range(S):
            # -------- PE --------
            pe_w = None
            if remote and s >= 1:
                pe_w = pwait(nc.tensor, rem_sem, 16 * s, pe_anchor[0])
            first_mm = None
            if s < T:
                m = nc.tensor.matmul(psG0[:], i64r[:], xp_sb[s % 3][:],
                                     start=True, stop=False)
                first_mm = m
                for c in range(NCH):
                    m = nc.tensor.matmul(psG0[:], h0stat(s, c),
                                         whh0[:, bass.ts(c, GS)],
                                         start=False, stop=(c == NCH - 1))
                    if pe_w is not None and c == 0:
                        add_dep_helper(m.ins, pe_w.ins, False, "h0 arrival")
                for c in range(NCH):
                    nc.tensor.matmul(psSK[:, 0:128], sk0stat(s, c),
                                     wh20[:, bass.ts(c, HS)],
                                     start=(c == 0), stop=(c == NCH - 1))
                nc.tensor.matmul(psM[:, 0:256], ones_sb[:], mrhs(s, 0),
                                 start=True, stop=True)
            if s >= 1:
                nc.tensor.matmul(psG1[:], i64r[:], b1_r[:],
                                 start=True, stop=False)
                for c in range(NCH):
                    m = nc.tensor.matmul(psG1[:], h0stat(s, c),
                                         wih1[:, bass.ts(c, GS)],
                                         start=False, stop=False)
                    if pe_w is not None and c == 0:
                        add_dep_helper(m.ins, pe_w.ins, False, "h0 arrival")
                for c in range(NCH):
                    nc.tensor.matmul(psG1[:], h1stat(s, c),
                                     whh1[:, bass.ts(c, GS)],
                                     start=False, stop=(c == NCH - 1))
                for c in range(NCH):
                    nc.tensor.matmul(psSK[:, 128:256], sk1stat(s, c),
                                     wh21[:, bass.ts(c, HS)],
                                     start=(c == 0), stop=(c == NCH - 1))
                nc.tensor.matmul(psM[:, 256:512], ones_sb[:], mrhs(s - 1, 1),
                                 start=True, stop=True)

            # -------- ACT + DVE: cells --------
            def cell(layer, gps, g_sb, c_state, skslice, skt, sks, msl,
                     hn, tt1, tt2, th, hc, uu1, uu2):
                nc.scalar.activation(g_sb[:, 0:384], gps[:, 0:384], AF.Sigmoid)
                nc.scalar.activation(g_sb[:, 384:512], gps[:, 384:512],
                                     AF.Tanh)
                nc.vector.tensor_add(skt[:], skslice,
                                     bh20_sb[:] if layer == 0 else bh21_sb[:])
                nc.scalar.activation(sks[:], skt[:], AF.Sigmoid)
                nc.vector.tensor_mul(tt1[:], g_sb[:, 128:256], c_state[:])
                nc.vector.tensor_mul(tt2[:], g_sb[:, 0:128],
                                     g_sb[:, 384:512])
                nc.vector.tensor_add(c_state[:], tt1[:], tt2[:])
                nc.scalar.activation(th[:], c_state[:], AF.Tanh)
                nc.vector.tensor_mul(hc[:], g_sb[:, 256:384], th[:])
                nc.vector.tensor_mul(uu1[:], hc[:], msl[0])
                nc.vector.tensor_mul(uu2[:], sks[:], msl[1])
                nc.vector.tensor_add(hn[:], uu1[:], uu2[:])

            if s < T:
                cell(0, psG0, g0_sb, c0_st, psSK[:, 0:128], ct["sk0t"],
                     ct["sk0s"], (psM[:, 0:128], psM[:, 128:256]), ct["hn0"],
                     ct["t1a"], ct["t2a"], ct["tha"], ct["hca"], ct["u1a"],
                     ct["u2a"])
            if s >= 1:
                cell(1, psG1, g1_sb, c1_st, psSK[:, 128:256], ct["sk1t"],
                     ct["sk1s"], (psM[:, 256:384], psM[:, 384:512]), ct["hn1"],
                     ct["t1b"], ct["t2b"], ct["thb"], ct["hcb"], ct["u1b"],
                     ct["u2b"])

            # -------- transposes + pair tile --------
            tr = None
            if s < T:
                tr = nc.tensor.transpose(psT[:, 0:64], ct["hn0"][:], i64f[:])
            if s >= 1:
                tr = nc.tensor.transpose(psT[:, 64:128], ct["hn1"][:], i64f[:])
            pe_anchor[0] = tr
            if remote and s >= 3:
                lw = pwait(nc.vector, loc_sem, 16 * (s - 2), dve_anchor[0])
                prc = nc.vector.tensor_copy(pr[s % 3][:], psT[:])
                add_dep_helper(prc.ins, lw.ins, False, "pr reuse")
            else:
                prc = nc.vector.tensor_copy(pr[s % 3][:], psT[:])
            dve_anchor[0] = prc

            # -------- exchange --------
            if remote:
                nc.gpsimd.remote_dma_broadcast(
                    hb[s % 3][:, bass.ds(rank * 128, 128)], pr[s % 3][:],
                    rem_sem, loc_sem,
                    rdests=[(0, j) for j in range(NCORES)])
                nc.gpsimd.trigger_dma(count=None)
            else:
                nc.sync.dma_start(cc_in[s % 2][:], pr[s % 3][:])
                nc.gpsimd.collective_compute(
                    "AllGather", ALU.bypass,
                    replica_groups=[list(range(NCORES))],
                    ins=[cc_in[s % 2].opt()], outs=[cc_out[s % 2].opt()])
                nc.sync.dma_start(
                    hb[s % 3][:].rearrange("p (c m) -> p c m", m=128),
                    cc_out[s % 2][:].rearrange("(c p) m -> p c m", p=128))

            # -------- bf16 cast of arrived round (s-1) --------
            if s >= 1:
                dw = None
                if remote:
                    dw = pwait(nc.vector, rem_sem, 16 * s, dve_anchor[0])
                cst = nc.vector.tensor_copy(hbb[(s - 1) % 3][:],
                                            hb[(s - 1) % 3][:])
                if dw is not None:
                    add_dep_helper(cst.ins, dw.ins, False, "cast arrival")

            # -------- SP DMAs --------
            if s + 2 < T:
                sp_emit(xp_load(s + 2))
            mi = mask_load(s + 2)
            if mi is not None:
                sp_emit(mi)
            if s >= 2:
                if remote:
                    sp_emit(pwait(nc.sync, rem_sem, 16 * s, sp_chain[0]))
                st = nc.sync.dma_start(
                    H1T_dram[s - 2].rearrange("p (c m) -> p c m", m=64),
                    hb[(s - 1) % 3][:].rearrange(
                        "p (c n) -> p c n", n=128)[:, :, 64:128])
                sp_emit(st)
            # phase A filler
            cidx = s // 2 + LOOKAHEAD
            if s % 2 == 0 and cidx < NCHUNK:
                phase_a_chunk(cidx)

        # final H1T store: row T-1 comes from round T
        if remote:
            sp_emit(pwait(nc.sync, rem_sem, 16 * (T + 1), sp_chain[0]))
        stf = nc.sync.dma_start(
            H1T_dram[T - 1].rearrange("p (c m) -> p c m", m=64),
            hb[T % 3][:].rearrange("p (c n) -> p c n", n=128)[:, :, 64:128])
        sp_emit(stf)

        # ---------- phase C: y = h1 @ WoutT + b ----------
        h1pair = [sb.tile([128, 1024], F32R, tag=f"h1p{i}", name=f"h1p{i}")
                  for i in range(2)]
        y_ev = [sb.tile([128, OUT], F32, tag=f"yev{i}", name=f"yev{i}")
                for i in range(2)]
        rank_sp = nc.sync.partition_id()
        for i in range(NT // 2):
            par = i % 2
            base = rank_sp * NT + 2 * i
            for j in range(2):
                sp_emit(nc.sync.dma_start(
                    h1pair[par][:].rearrange("p (c m) -> p c m", m=128)
                    [:, :, j * 64:(j + 1) * 64],
                    H1T_dram[bass.ds(base + j, 1)].rearrange(
                        "o p (c m) -> (o p) c m", m=64)))
            nc.tensor.matmul(psA[par][:, 0:OUT], i128r[:], bo_r[:],
                             start=True, stop=False)
            for k in range(NCH):
                nc.tensor.matmul(psA[par][:, 0:OUT],
                                 h1pair[par][:, bass.ts(k, 128)],
                                 wout[:, bass.ts(k, OUT)],
                                 start=False, stop=(k == NCH - 1))
            nc.vector.tensor_copy(y_ev[par][:], psA[par][:, 0:OUT])
            sp_emit(nc.sync.dma_start(
                y_out[bass.ts(i, 2)].rearrange("t b d -> (t b) d"),
                y_ev[par][:]))

    # patch placeholder waits
    for (wi, val) in patches:
        ow = wi.ins.sync_info.on_wait
        assert len(ow) == 1, ow
        # the placeholder semaphore wait is the one we created first
        ow[0].wait_value = val

    nc.compile()
    return nc


# ================= host-side wrapper =================

def _prep_inputs(inputs, T):
    """Slice/transpose/pack full inputs per core. Returns list of in_maps."""
    f32 = np.float32
    bf16 = ml_dtypes.bfloat16
    targets = np.asarray(inputs["targets"], f32)[:T]
    h0 = np.asarray(inputs["h0"], f32)
    c0 = np.asarray(inputs["c0"], f32)
    codes = np.asarray(inputs["mask_codes"])[:T]
    mW1 = ((codes == 0) | (codes == 2)).astype(f32)   # [T, 2, H]
    mW2 = ((codes == 1) | (codes == 2)).astype(f32)

    W = {k: np.asarray(inputs[k], f32) for k in
         ["W_ih0", "W_hh0", "b_ih0", "b_hh0", "W_ih1", "W_hh1", "b_ih1",
          "b_hh1", "Wh2_0", "bh2_0", "Wh2_1", "bh2_1", "W_out", "b_out"]}

    in_maps = []
    for r in range(NCORES):
        hs = slice(128 * r, 128 * r + 128)
        # gate rows for this core in [i|f|o|g] order
        rows = np.concatenate(
            [GATE_PERM[b] * H + np.arange(128 * r, 128 * r + 128)
             for b in range(4)])

        def wT(Wm):   # [4H, K] -> [K, 512] slice, chunked [128, 8*512]
            sl = Wm[rows].T.astype(f32)           # [K, 512]
            K = sl.shape[0]
            return np.ascontiguousarray(
                sl.reshape(K // 128, 128, GS).transpose(1, 0, 2).reshape(
                    128, (K // 128) * GS))

        def colchunk(Mat, width):  # [K, width] -> [128, (K/128)*width]
            K = Mat.shape[0]
            return np.ascontiguousarray(
                Mat.reshape(K // 128, 128, width).transpose(1, 0, 2).reshape(
                    128, (K // 128) * width))

        bias0 = (W["b_ih0"] + W["b_hh0"])[rows]
        bias1 = (W["b_ih1"] + W["b_hh1"])[rows]

        h0T = colchunk(h0[0].T[:, :].astype(f32), 64)   # [1024,64]->[128,512]
        h1T = colchunk(h0[1].T[:, :].astype(f32), 64)

        # masks: [T,2,256] = [m1_slice | m2_slice]
        mk = np.concatenate([mW1[:, :, hs], mW2[:, :, hs]], axis=2)

        d = {
            "tgt": targets,
            "Whh0T": wT(W["W_hh0"]),
            "Wih1T": wT(W["W_ih1"]),
            "Whh1T": wT(W["W_hh1"]),
            "Wih0T": wT(W["W_ih0"]),
            "WoutT": colchunk(W["W_out"].T.astype(f32), OUT),
            "Wh20b": colchunk(W["Wh2_0"][:, hs], HS).astype(bf16),
            "Wh21b": colchunk(W["Wh2_1"][:, hs], HS).astype(bf16),
            "b0bc": np.tile(bias0, (128, 1)).astype(f32),
            "b1bc": np.tile(bias1, (B, 1)).astype(f32),
            "bh20": np.tile(W["bh2_0"][hs], (B, 1)).astype(f32),
            "bh21": np.tile(W["bh2_1"][hs], (B, 1)).astype(f32),
            "bobc": np.tile(W["b_out"], (128, 1)).astype(f32),
            "I64d": np.eye(64, dtype=f32),
            "I64r_d": np.eye(64, dtype=f32),
            "I128d": np.eye(128, dtype=f32),
            "I128r_d": np.eye(128, dtype=f32),
            "onesd": np.ones((1, 64), bf16),
            "h0Ti": h0T, "h1Ti": h1T,
            "h0Tib": h0T.astype(bf16), "h1Tib": h1T.astype(bf16),
            "c00": np.ascontiguousarray(c0[0][:, hs]),
            "c01": np.ascontiguousarray(c0[1][:, hs]),
            "maskd": np.ascontiguousarray(mk).astype(bf16).reshape(1, T * 512),
        }
        in_maps.append(d)
    return in_maps


def run(inputs, T=None, mode=None, time_holder=None):
    from concourse.bass_utils import run_bass_kernel_spmd
    if T is None:
        T = np.asarray(inputs["targets"]).shape[0]
    if mode is None:
        mode = os.environ.get("DEC_MODE", "remote")
    key = (T, mode)
    if key not in _BUILD_CACHE:
        t0 = time.time()
        _BUILD_CACHE[key] = build(T, mode)
        print(f"[kernel] bass build {key}: {time.time()-t0:.1f}s", flush=True)
    nc = _BUILD_CACHE[key]
    in_maps = _prep_inputs(inputs, T)
    t0 = time.time()
    res = run_bass_kernel_spmd(nc, in_maps, core_ids=list(range(NCORES)))
    wall = time.time() - t0
    if time_holder is not None:
        time_holder.append(wall)
    NT = T // NCORES
    y = np.empty((T, B, OUT), np.float32)
    for r in range(NCORES):
        y[r * NT:(r + 1) * NT] = res.results[r]["y_out"]
    return y


def run_sim(inputs, T, mode="collective"):
    """MultiCoreSim correctness check (CPU)."""
    import concourse.libnrt as libnrt
    libnrt.get_trn2_nc_mapping = lambda: {
        (d, i): i for d in range(16) for i in range(8)}
    libnrt.nc_to_real_nc.cache_clear()
    import concourse.bass_interp as bim
    bim.get_device_id_to_routing_id_mapping = lambda: {d: d for d in range(16)}
    from concourse.bass_interp import MultiCoreSim
    nc = build(T, mode)
    nc.detect_race_conditions = False
    nc.m.detect_race_conditions = False
    in_maps = _prep_inputs(inputs, T)
    sim = MultiCoreSim(nc, num_cores=NCORES)
    for c in range(NCORES):
        for k, v in in_maps[c].items():
            sim.cores[c].tensor(k)[:] = v
    sim.simulate(check_with_hw=False)
    NT = T // NCORES
    y = np.empty((T, B, OUT), np.float32)
    for r in range(NCORES):
        y[r * NT:(r + 1) * NT] = np.asarray(
            sim.cores[r].mem_tensor("y_out")).reshape(NT, B, OUT)
    return y


_EXEC_CACHE = {}


def run_timed(inputs, T=None, mode=None, reps=3):
    """Like run(), but keeps inputs device-resident and the jitted executable
    cached, so repeat executions time the device work (not H2D of 1.6GB)."""
    import jax
    from jax.sharding import Mesh, PartitionSpec
    from jax.experimental.shard_map import shard_map
    from concourse import bass2jax
    import concourse.mybir as mb

    if T is None:
        T = np.asarray(inputs["targets"]).shape[0]
    if mode is None:
        mode = os.environ.get("DEC_MODE", "collective")
    key = (T, mode)
    if key not in _BUILD_CACHE:
        t0 = time.time()
        _BUILD_CACHE[key] = build(T, mode)
        print(f"[kernel] bass build {key}: {time.time()-t0:.1f}s", flush=True)
    nc = _BUILD_CACHE[key]
    in_maps = _prep_inputs(inputs, T)

    if key not in _EXEC_CACHE:
        bass2jax.install_neuronx_cc_hook()
        partition_name = (nc.partition_id_tensor.name
                          if nc.partition_id_tensor else None)
        in_names, out_names, out_avals, zero_shapes = [], [], [], []
        for alloc in nc.m.functions[0].allocations:
            if not isinstance(alloc, mb.MemoryLocationSet):
                continue
            name = alloc.memorylocations[0].name
            if alloc.kind == "ExternalInput":
                if name != partition_name:
                    in_names.append(name)
            elif alloc.kind == "ExternalOutput":
                dt_np = mb.dt.np(alloc.dtype)
                out_avals.append(jax.core.ShapedArray(
                    tuple(alloc.tensor_shape), dt_np))
                out_names.append(name)
                zero_shapes.append((tuple(alloc.tensor_shape), dt_np))
        n_params = len(in_names)
        all_in_names = list(in_names) + list(out_names)
        if partition_name is not None:
            all_in_names.append(partition_name)
        donate = tuple(range(n_params, n_params + len(out_names)))

        def _body(*args):
            operands = list(args)
            if partition_name is not None:
                operands.append(bass2jax.partition_id_tensor())
            outs = bass2jax._bass_exec_p.bind(
                *operands,
                out_avals=tuple(out_avals),
                in_names=tuple(all_in_names),
                out_names=tuple(out_names),
                lowering_input_output_aliases=(),
                sim_require_finite=True,
                sim_require_nnan=True,
                nc=nc,
            )
            return tuple(outs)

        devices = jax.devices()[:NCORES]
        mesh = Mesh(np.asarray(devices), ("core",))
        n_outs = len(out_names)
        sharded = jax.jit(
            shard_map(_body, mesh=mesh,
                      in_specs=(PartitionSpec("core"),) * (n_params + n_outs),
                      out_specs=(PartitionSpec("core"),) * n_outs,
                      check_rep=False),
            donate_argnums=donate, keep_unused=True)
        _EXEC_CACHE[key] = (sharded, in_names, out_names, zero_shapes, mesh)

    sharded, in_names, out_names, zero_shapes, mesh = _EXEC_CACHE[key]
    from jax.sharding import NamedSharding, PartitionSpec as P
    shard = NamedSharding(mesh, P("core"))
    concat_in = [
        jax.device_put(np.concatenate(
            [np.asarray(in_maps[c][n]) for c in range(NCORES)], axis=0), shard)
        for n in in_names]
    for a in concat_in:
        a.block_until_ready()
    walls = []
    out_arrs = None
    for rep in range(reps):
        zeros = [jax.device_put(
            np.zeros((NCORES * sh[0], *sh[1:]), dt), shard)
            for (sh, dt) in zero_shapes]
        for z in zeros:
            z.block_until_ready()
        t0 = time.time()
        out_arrs = sharded(*concat_in, *zeros)
        for o in out_arrs:
            o.block_until_ready()
        walls.append(time.time() - t0)
        print(f"[timed] rep {rep}: {walls[-1]*1e3:.2f} ms", flush=True)
    NT = T // NCORES
    yi = out_names.index("y_out")
    yall = np.asarray(out_arrs[yi]).reshape(NCORES, NT, B, OUT)
    y = yall.reshape(T, B, OUT)
    return y, min(walls)


def kernel(**inputs):
    return run(inputs)
